# revision 63
# baseline (speedup 1.0000x reference)
"""Trainium2 Bass kernel for MHSA with Transformer-XL relative position bias.

Problem: B=16, T=1024, DM=256, H=4, HS=64 fp32.
Sharding: pure data-parallel over batch across 8 cores (2 batches/core).

Per-core pipeline (M = 2*1024 = 2048 rows), per (b,h) software-pipelined in
3 stages (X-scores / C-scores+softmax / AV):
  1. LN in [m, d] layout (bn_stats), PE-transpose -> xnT/posT [256, M] bf16
  2. Projections via PE: QuT/QvT/KT/PT [256, M] bf16 (s on partitions),
     V [M, 256] bf16
  3. Per (b, h):
     - X = QvT.T @ PT -> PSUM -> fp8e4 into xw [128, 8, 1025] (col 0 zero),
       ONE batched DMA to DRAM scratch [1024*1025] fp8;
       rel_shift == flat-buffer shear: ONE batched re-read [128, 8, 1024]
       with row stride 1024 from element offset 1024.
     - C = QuT.T @ KT into PSUM; R(fp8) added into the same PSUM via
       identity matmul (PE); exp((C+R)/8) on ACT straight from PSUM with
       fused row-sum (logits small, no max subtraction); A = E*(1/S) in
       place (DVE 4x); ONE batched xbar-transpose A [128, 8192] ->
       at [128, 64, 128] (g = mt*8+nt).
     - AV^T accumulated on PE from at (strided g-slices give N=512 rhs).
  4. Out-proj from AVT, + residual (DVE) + bo (Pool), DMA out.
"""
import sys

sys.path.insert(0, "/opt/trn_rl_repo")

import numpy as np

import concourse.bass as bass
import concourse.bacc as bacc
import concourse.tile as tile
from concourse import mybir
from concourse.masks import make_identity
from concourse.bass_utils import run_bass_kernel_spmd

B, T, DM, H, HS = 16, 1024, 256, 4, 64
NCORES = 8
BL = B // NCORES          # local batches per core
M = BL * T                # local rows (2048)
NMT = M // 128            # m-tiles (16)
P = 128
NBH = BL * H              # (b, h) pairs per core (8)
LN_EPS = 1e-3
F32 = mybir.dt.float32
BF16 = mybir.dt.bfloat16
FP8 = mybir.dt.float8e4


def build_bass():
    nc = bacc.Bacc("TRN2", target_bir_lowering=False, debug=False,
                   enable_asserts=False, num_devices=NCORES)

    x_in = nc.dram_tensor("x", [M, DM], F32, kind="ExternalInput").ap()
    pos_in = nc.dram_tensor("pos", [M, DM], F32, kind="ExternalInput").ap()
    wq_in = nc.dram_tensor("wq", [DM, DM], F32, kind="ExternalInput").ap()
    wk_in = nc.dram_tensor("wk", [DM, DM], F32, kind="ExternalInput").ap()
    wv_in = nc.dram_tensor("wv", [DM, DM], F32, kind="ExternalInput").ap()
    wp_in = nc.dram_tensor("wp", [DM, DM], F32, kind="ExternalInput").ap()
    wo_in = nc.dram_tensor("wo", [DM, DM], F32, kind="ExternalInput").ap()
    bqu_in = nc.dram_tensor("bqu", [DM], F32, kind="ExternalInput").ap()
    bqv_in = nc.dram_tensor("bqv", [DM], F32, kind="ExternalInput").ap()
    bk_in = nc.dram_tensor("bk", [DM], F32, kind="ExternalInput").ap()
    bo_in = nc.dram_tensor("bo", [DM], F32, kind="ExternalInput").ap()
    out = nc.dram_tensor("out", [M, DM], F32, kind="ExternalOutput").ap()

    # scratch split in half (tiles 0-4 / 4-7, tile 4 written to both) so each
    # shear re-read half depends only on its own write half
    scrA = [
        nc.dram_tensor(f"xscra{i}", [641 * (T + 1)], FP8, kind="Internal").ap()
        for i in range(2)
    ]
    scrB = [
        nc.dram_tensor(f"xscrb{i}", [512 * (T + 1)], FP8, kind="Internal").ap()
        for i in range(2)
    ]

    with tile.TileContext(nc) as tc:
        with tc.tile_pool(name="persist", bufs=1) as pp:
            # --- persistent SBUF ---
            ident = pp.tile([P, P], F32)
            make_identity(nc, ident)
            ident_bf = pp.tile([P, P], BF16)
            nc.gpsimd.tensor_copy(out=ident_bf, in_=ident)

            def load_w(ap_in, name):
                tmp = pp.tile([P, 2, DM], F32, tag=f"{name}tmp", name=f"{name}tmp")
                nc.sync.dma_start(
                    out=tmp,
                    in_=bass.AP(tensor=ap_in.tensor, offset=0,
                                ap=[[DM, P], [P * DM, 2], [1, DM]]))
                ts = [pp.tile([P, DM], BF16, tag=f"{name}{c}", name=f"{name}{c}")
                      for c in range(2)]
                for c in range(2):
                    nc.gpsimd.tensor_copy(out=ts[c], in_=tmp[:, c, :])
                return ts

            wq_sb = load_w(wq_in, "wq")
            wk_sb = load_w(wk_in, "wk")
            wv_sb = load_w(wv_in, "wv")
            wp_sb = load_w(wp_in, "wp")
            wo_sb = load_w(wo_in, "wo")

            def load_col(ap_in, name):
                ts = [pp.tile([P, 1], F32, tag=f"{name}{c}", name=f"{name}{c}") for c in range(2)]
                for c in range(2):
                    nc.sync.dma_start(
                        out=ts[c],
                        in_=bass.AP(tensor=ap_in.tensor, offset=c * P, ap=[[1, P], [1, 1]]),
                    )
                return ts

            bqu_c = load_col(bqu_in, "bqu")
            bqv_c = load_col(bqv_in, "bqv")
            bk_c = load_col(bk_in, "bk")

            bo_b = pp.tile([P, DM], F32, tag="bo_b", name="bo_b")
            nc.sync.dma_start(
                out=bo_b,
                in_=bass.AP(tensor=bo_in.tensor, offset=0, ap=[[0, P], [1, DM]]),
            )

            eps_t = pp.tile([P, 1], F32)
            nc.vector.memset(eps_t, LN_EPS)

            x_res = pp.tile([P, NMT, DM], F32)        # residual copy of inputs
            xnT3 = pp.tile([P, 2, M], BF16, tag="xnT3", name="xnT3")
            posT3 = pp.tile([P, 2, M], BF16, tag="posT3", name="posT3")
            quT = [pp.tile([P, M], BF16, tag=f"quT{c}", name=f"quT{c}") for c in range(2)]
            qvT = [pp.tile([P, M], BF16, tag=f"qvT{c}", name=f"qvT{c}") for c in range(2)]
            kT = [pp.tile([P, M], BF16, tag=f"kT{c}", name=f"kT{c}") for c in range(2)]
            pT = [pp.tile([P, M], BF16, tag=f"pT{c}", name=f"pT{c}") for c in range(2)]
            v_sb = pp.tile([P, NMT, DM], BF16)        # V[mt*128+p, s] at [:, mt, s]
            avT = [pp.tile([P, M], BF16, tag=f"avT{c}", name=f"avT{c}") for c in range(2)]

            # phase-3 double buffers
            xw = [pp.tile([P, 8, T + 1], FP8, tag=f"xw{c}", name=f"xw{c}") for c in range(2)]
            rbf = [pp.tile([P, 8, T], FP8, tag=f"rbf{c}", name=f"rbf{c}") for c in range(2)]
            a_sb = [pp.tile([P, 8, T], BF16, tag=f"asb{c}", name=f"asb{c}") for c in range(2)]
            at = [pp.tile([P, 64, P], BF16, tag=f"at{c}", name=f"at{c}") for c in range(2)]
            for c in range(2):
                for mt in range(8):
                    nc.gpsimd.memset(xw[c][:, mt, 0:1], 0.0)

            # first input group loads up-front; later groups stream in the loop
            nc.sync.dma_start(
                out=x_res[:, 0:4, :],
                in_=bass.AP(tensor=x_in.tensor, offset=0,
                            ap=[[DM, P], [P * DM, 4], [1, DM]]))

            # ---------------- phase 1+2: LN + transposes + projections ----------------
            with tc.tile_pool(name="ph1", bufs=3) as sb1, \
                 tc.tile_pool(name="pos1", bufs=2) as sbp, \
                 tc.tile_pool(name="ps1", bufs=2, space="PSUM") as ps1, \
                 tc.tile_pool(name="ps2", bufs=1, space="PSUM") as ps2, \
                 tc.tile_pool(name="ps2v", bufs=2, space="PSUM") as ps2v, \
                 tc.tile_pool(name="psE", bufs=1, space="PSUM") as psE:
                for g in range(4):
                    pt4 = sbp.tile([P, 4, DM], F32, tag="pt4")
                    nc.sync.dma_start(
                        out=pt4,
                        in_=bass.AP(tensor=pos_in.tensor, offset=4 * g * P * DM,
                                    ap=[[DM, P], [P * DM, 4], [1, DM]]))
                    if g < 3:
                        nc.sync.dma_start(
                            out=x_res[:, 4 * (g + 1):4 * (g + 2), :],
                            in_=bass.AP(tensor=x_in.tensor,
                                        offset=4 * (g + 1) * P * DM,
                                        ap=[[DM, P], [P * DM, 4], [1, DM]]))
                    mvg = sb1.tile([P, 4, 2], F32, tag="mvg")
                    for k in range(4):
                        mt = 4 * g + k
                        stats = sb1.tile([P, 6], F32, tag="stats")
                        nc.vector.bn_stats(out=stats, in_=x_res[:, mt, :])
                        nc.vector.bn_aggr(out=mvg[:, k, :], in_=stats)
                    rstd4 = sb1.tile([P, 4], F32, tag="rstd4")
                    nc.scalar.activation(out=rstd4, in_=mvg[:, :, 1:2],
                                         func=mybir.ActivationFunctionType.Sqrt,
                                         bias=eps_t, scale=1.0)
                    nc.vector.reciprocal(out=rstd4, in_=rstd4)
                    for k in range(4):
                        mt = 4 * g + k
                        xn = sb1.tile([P, DM], F32, tag="xn")
                        nc.vector.tensor_scalar(out=xn, in0=x_res[:, mt, :],
                                                scalar1=mvg[:, k, 0:1],
                                                scalar2=rstd4[:, k:k + 1],
                                                op0=mybir.AluOpType.subtract,
                                                op1=mybir.AluOpType.mult)
                        tp = ps1.tile([P, 2, P], F32, tag="tp")
                        tp2 = ps1.tile([P, 2, P], F32, tag="tp")
                        for c in range(2):
                            nc.tensor.transpose(tp[:, c, :], xn[:, c * P:(c + 1) * P], ident)
                            nc.tensor.transpose(tp2[:, c, :], pt4[:, k, c * P:(c + 1) * P], ident)
                        nc.scalar.copy(out=xnT3[:, :, mt * P:(mt + 1) * P], in_=tp)
                        nc.scalar.copy(out=posT3[:, :, mt * P:(mt + 1) * P], in_=tp2)
                    # projections for the m-chunk this group just produced
                    mc = g
                    msl = slice(mc * 512, (mc + 1) * 512)
                    for sc in range(2):
                        pq = ps2.tile([P, 512], F32, tag="pq")
                        pk = ps2.tile([P, 512], F32, tag="pk")
                        pps = ps2.tile([P, 512], F32, tag="pp")
                        for dc in range(2):
                            nc.tensor.matmul(pq, lhsT=wq_sb[dc][:, sc * P:(sc + 1) * P],
                                             rhs=xnT3[:, dc, msl],
                                             start=(dc == 0), stop=(dc == 1))
                            nc.tensor.matmul(pk, lhsT=wk_sb[dc][:, sc * P:(sc + 1) * P],
                                             rhs=xnT3[:, dc, msl],
                                             start=(dc == 0), stop=(dc == 1))
                            nc.tensor.matmul(pps, lhsT=wp_sb[dc][:, sc * P:(sc + 1) * P],
                                             rhs=posT3[:, dc, msl],
                                             start=(dc == 0), stop=(dc == 1))
                        nc.scalar.activation(out=quT[sc][:, msl], in_=pq,
                                             func=mybir.ActivationFunctionType.Identity,
                                             bias=bqu_c[sc], scale=1.0)
                        nc.vector.tensor_scalar_add(out=qvT[sc][:, msl], in0=pq,
                                                    scalar1=bqv_c[sc])
                        nc.scalar.activation(out=kT[sc][:, msl], in_=pk,
                                             func=mybir.ActivationFunctionType.Identity,
                                             bias=bk_c[sc], scale=1.0)
                        nc.scalar.copy(out=pT[sc][:, msl], in_=pps)
                    for k in range(4):
                        mt = 4 * g + k
                        pv = ps2v.tile([P, DM], F32, tag="pv")
                        for dc in range(2):
                            nc.tensor.matmul(pv, lhsT=xnT3[:, dc, mt * P:(mt + 1) * P],
                                             rhs=wv_sb[dc],
                                             start=(dc == 0), stop=(dc == 1))
                        nc.vector.tensor_copy(out=v_sb[:, mt, :], in_=pv)
                    if g == 1:
                        # hide attention step 0 (X scores for bh 0 + scratch
                        # write + shear-read prefetch) inside the fill: its
                        # inputs (qvT/pT chunks for batch 0) are ready now
                        for mt in range(8):
                            mg0 = slice(mt * P, (mt + 1) * P)
                            for nck in range(2):
                                xp0 = psE.tile([P, 512], F32, tag="xe")
                                nc.tensor.matmul(
                                    xp0, lhsT=qvT[0][0:64, mg0],
                                    rhs=pT[0][0:64, nck * 512:(nck + 1) * 512],
                                    start=True, stop=True)
                                dst0 = xw[0][:, mt, 1 + nck * 512:1 + (nck + 1) * 512]
                                if (mt * 2 + nck) % 8 == 0:
                                    nc.scalar.copy(out=dst0, in_=xp0)
                                else:
                                    nc.vector.tensor_copy(out=dst0, in_=xp0)
                            if mt == 4:
                                nc.gpsimd.dma_start(
                                    out=bass.AP(tensor=scrA[0].tensor, offset=0,
                                                ap=[[T + 1, P], [P * (T + 1), 5], [1, T + 1]]),
                                    in_=xw[0][:, 0:5, :])
                                nc.sync.dma_start(
                                    out=rbf[0][:, 0:4, :],
                                    in_=bass.AP(tensor=scrA[0].tensor, offset=T,
                                                ap=[[T, P], [P * T, 4], [1, T]]))
                        nc.gpsimd.dma_start(
                            out=bass.AP(tensor=scrB[0].tensor, offset=0,
                                        ap=[[T + 1, P], [P * (T + 1), 4], [1, T + 1]]),
                            in_=xw[0][:, 4:8, :])
                        nc.sync.dma_start(
                            out=rbf[0][:, 4:8, :],
                            in_=bass.AP(tensor=scrB[0].tensor, offset=T - 512,
                                        ap=[[T, P], [P * T, 4], [1, T]]))

            # ---------------- phase 3: attention per (b, h), pipelined ----------------
            with tc.tile_pool(name="sb3", bufs=6) as sb3, \
                 tc.tile_pool(name="psX", bufs=2, space="PSUM") as psX, \
                 tc.tile_pool(name="psC", bufs=2, space="PSUM") as psC, \
                 tc.tile_pool(name="psAV", bufs=2, space="PSUM") as psAV:
                avp = None

                def stage_b(sbh, mt):
                    b, h = divmod(sbh, H)
                    hh, po = h // 2, (h % 2) * 64
                    ssl = slice(po, po + 64)
                    mg = slice(b * T + mt * P, b * T + (mt + 1) * P)
                    cm = psC.tile([P, 1024], F32, tag="cm")
                    for nck in range(2):
                        nc.tensor.matmul(
                            cm[:, nck * 512:(nck + 1) * 512],
                            lhsT=quT[hh][ssl, mg],
                            rhs=kT[hh][ssl, b * T + nck * 512:b * T + (nck + 1) * 512],
                            start=True, stop=False)
                        nc.tensor.matmul(
                            cm[:, nck * 512:(nck + 1) * 512],
                            lhsT=ident_bf,
                            rhs=rbf[sbh % 2][:, mt, nck * 512:(nck + 1) * 512],
                            start=False, stop=True)
                    sinv = sb3.tile([P, 1], F32, tag="sinv")
                    nc.scalar.activation(
                        out=a_sb[sbh % 2][:, mt, :],
                        in_=cm, func=mybir.ActivationFunctionType.Exp,
                        scale=0.125, accum_out=sinv)
                    nc.vector.reciprocal(out=sinv, in_=sinv)
                    nc.vector.tensor_scalar_mul(
                        out=a_sb[sbh % 2][:, mt, :],
                        in0=a_sb[sbh % 2][:, mt, :], scalar1=sinv)

                def emit_ph4(mt):
                    op = psX.tile([P, 512], F32, tag="xp")
                    for sc in range(2):
                        nc.tensor.matmul(op[:, 0:DM],
                                         lhsT=avT[sc][:, mt * P:(mt + 1) * P],
                                         rhs=wo_sb[sc],
                                         start=(sc == 0), stop=(sc == 1))
                    ot = sb3.tile([P, DM], F32, tag="ot")
                    nc.vector.scalar_tensor_tensor(out=ot, in0=op[:, 0:DM],
                                                   scalar=0.0,
                                                   in1=x_res[:, mt, :],
                                                   op0=mybir.AluOpType.bypass,
                                                   op1=mybir.AluOpType.add)
                    ot2 = sb3.tile([P, DM], F32, tag="ot2")
                    nc.gpsimd.tensor_tensor(out=ot2, in0=ot, in1=bo_b,
                                            op=mybir.AluOpType.add)
                    nc.sync.dma_start(out=out[mt * P:(mt + 1) * P, :], in_=ot2)

                for step in range(1, NBH + 3):
                    sa, sb_, st_, sc_ = step, step - 1, step - 2, step - 3
                    if step == NBH:
                        # batch-0 rows of the out-projection: their avT inputs
                        # retired at step NBH-2; fills the pipeline drain
                        for mt in range(8):
                            emit_ph4(mt)
                    if 0 <= st_ < NBH:
                        # transpose A(st_): scaled during previous step,
                        # consumed by AV next step -> a full step of slack.
                        # The final one is split so AV's m<512 matmuls can
                        # start after the first half.
                        if st_ == NBH - 1:
                            nc.sync.dma_start_transpose(
                                out=at[st_ % 2][:, 0:32, :],
                                in_=a_sb[st_ % 2][:, 0:4, :])
                            nc.sync.dma_start_transpose(
                                out=at[st_ % 2][:, 32:64, :],
                                in_=a_sb[st_ % 2][:, 4:8, :])
                        else:
                            nc.sync.dma_start_transpose(out=at[st_ % 2], in_=a_sb[st_ % 2])
                    if 0 <= sc_ < NBH:
                        avp = psAV.tile([P, 512], F32, tag="avp")
                    for mt in range(8):
                        if 0 <= sc_ < NBH:
                            b, h = divmod(sc_, H)
                            # mc-major: two nt-chunks per loop slot so the
                            # m<512 accumulation finishes (and drains) while
                            # the m>=512 half is still accumulating
                            for u in range(2):
                                nt = (2 * mt + u) % 8
                                mc = (2 * mt + u) // 8
                                nc.tensor.matmul(
                                    avp[64 * mc:64 * (mc + 1), :],
                                    lhsT=v_sb[:, b * 8 + nt, h * HS:(h + 1) * HS],
                                    rhs=at[sc_ % 2][:, 32 * mc + nt:32 * mc + nt + 25:8, :],
                                    start=(nt == 0), stop=(nt == 7))
                            if mt == 3:
                                hh, po = h // 2, (h % 2) * 64
                                nc.scalar.copy(
                                    out=avT[hh][po:po + 64, b * T:b * T + 512],
                                    in_=avp[0:64, :])
                        if sa < NBH:
                            b, h = divmod(sa, H)
                            hh, po = h // 2, (h % 2) * 64
                            ssl = slice(po, po + 64)
                            mg = slice(b * T + mt * P, b * T + (mt + 1) * P)
                            for nck in range(2):
                                xp = psX.tile([P, 512], F32, tag="xp")
                                nc.tensor.matmul(
                                    xp, lhsT=qvT[hh][ssl, mg],
                                    rhs=pT[hh][ssl, b * T + nck * 512:b * T + (nck + 1) * 512],
                                    start=True, stop=True)
                                dst = xw[sa % 2][:, mt, 1 + nck * 512:1 + (nck + 1) * 512]
                                if (mt * 2 + nck) % 8 == 0:
                                    nc.scalar.copy(out=dst, in_=xp)
                                else:
                                    nc.vector.tensor_copy(out=dst, in_=xp)
                        if 0 <= sb_ < NBH and mt >= 4:
                            stage_b(sb_, mt - 4)
                        if sa < NBH and mt == 4:
                            # write half A (x-tiles 0-4) and prefetch shear
                            # re-read of rows 0-511 (needs padded rows 0-512)
                            nc.gpsimd.dma_start(
                                out=bass.AP(tensor=scrA[sa % 2].tensor, offset=0,
                                            ap=[[T + 1, P], [P * (T + 1), 5], [1, T + 1]]),
                                in_=xw[sa % 2][:, 0:5, :])
                            nc.sync.dma_start(
                                out=rbf[sa % 2][:, 0:4, :],
                                in_=bass.AP(tensor=scrA[sa % 2].tensor, offset=T,
                                            ap=[[T, P], [P * T, 4], [1, T]]))
                    if 0 <= sb_ < NBH:
                        stage_b(sb_, 4)
                        stage_b(sb_, 5)
                    if sa < NBH:
                        # write half B (x-tiles 4-7; tile 4 duplicated) and
                        # re-read rows 512-1023 (padded rows 512-1023)
                        nc.gpsimd.dma_start(
                            out=bass.AP(tensor=scrB[sa % 2].tensor, offset=0,
                                        ap=[[T + 1, P], [P * (T + 1), 4], [1, T + 1]]),
                            in_=xw[sa % 2][:, 4:8, :])
                        nc.sync.dma_start(
                            out=rbf[sa % 2][:, 4:8, :],
                            in_=bass.AP(tensor=scrB[sa % 2].tensor, offset=T - 512,
                                        ap=[[T, P], [P * T, 4], [1, T]]))
                    if 0 <= sb_ < NBH:
                        stage_b(sb_, 6)
                        stage_b(sb_, 7)
                    if 0 <= sc_ < NBH:
                        b, h = divmod(sc_, H)
                        hh, po = h // 2, (h % 2) * 64
                        nc.scalar.copy(
                            out=avT[hh][po:po + 64, b * T + 512:b * T + 1024],
                            in_=avp[64:128, :])

                for mt in range(8, NMT):
                    emit_ph4(mt)
    nc.finalize()
    return nc



_NC = None


def make_in_maps(inputs):
    f = lambda a: np.ascontiguousarray(np.asarray(a, dtype=np.float32))
    x = f(inputs["inputs"]).reshape(B, T, DM)
    pos = f(inputs["pos_enc"]).reshape(B, T, DM)
    wq0 = f(inputs["Wq"]).reshape(DM, DM)
    wk0 = f(inputs["Wk"]).reshape(DM, DM)
    wv0 = f(inputs["Wv"]).reshape(DM, DM)
    wp = f(inputs["Wp"]).reshape(DM, DM)
    wo = f(inputs["Wo"]).reshape(DM, DM)
    gamma = f(inputs["gamma"]).reshape(DM, 1)
    beta = f(inputs["beta"]).reshape(DM)
    # fold LN's gamma into the x-side weights, beta into the projection biases,
    # and bv through softmax (rows sum to 1) into the output bias
    wq, wk, wv = gamma * wq0, gamma * wk0, gamma * wv0
    bqu = (f(inputs["bq"]).reshape(DM) + f(inputs["pos_bias_u"]).reshape(DM)
           + beta @ wq0)
    bqv = (f(inputs["bq"]).reshape(DM) + f(inputs["pos_bias_v"]).reshape(DM)
           + beta @ wq0)
    bk = f(inputs["bk"]).reshape(DM) + beta @ wk0
    bv_eff = f(inputs["bv"]).reshape(DM) + beta @ wv0
    bo = f(inputs["bo"]) + bv_eff @ wo
    shared = dict(
        wq=wq, wk=wk, wv=wv, wp=wp, wo=wo,
        bqu=bqu, bqv=bqv, bk=bk, bo=bo,
    )
    in_maps = []
    for c in range(NCORES):
        sl = slice(c * BL, (c + 1) * BL)
        in_maps.append(dict(
            x=np.ascontiguousarray(x[sl].reshape(M, DM)),
            pos=np.ascontiguousarray(pos[sl].reshape(M, DM)),
            **shared,
        ))
    return in_maps


def kernel(**inputs) -> np.ndarray:
    global _NC
    if _NC is None:
        _NC = build_bass()
    in_maps = make_in_maps(inputs)
    res = run_bass_kernel_spmd(_NC, in_maps, core_ids=list(range(NCORES)))
    outs = [r["out"].reshape(BL, T, DM) for r in res.results]
    return np.concatenate(outs, axis=0)
